# revision 28
# baseline (speedup 1.0000x reference)
"""FAPE loss kernel for Trainium2 (8 NeuronCores, Bass/Tile).

Math
----
The reference computes, for frames i and residue-atoms (l, j):

    local[i, lj, d] = sum_c coords[lj, c] * R[i, d, c] - off[i, d]
    d2[i, lj]       = sum_d (pred_local - true_local)^2
    loss            = sum_{i,lj} m[i] * m[l] * min(sqrt(d2 + eps), 10) / ((sum m)^2 * 3 + eps) / 10

The delta is linear in the 7-vector u'[lj] = [pred_coords(3), true_coords(3), 1]:
    delta_d[i, lj] = dot(u'[lj], w_d[i]),  w_d[i] = [pR[i,d,:], -tR[i,d,:], -(offp-offt)[i,d]]
so d2 is a quadratic form
    d2[i, lj] = sum_{a<=b} mult_ab * u'_a u'_b * Q[i,(a,b)],  Q[i] = sum_d w_d w_d^T

Host (O(L) work): builds P[28, 6144] = pairwise products of u' (residue mask folded
in as zeroed columns, so masked entries give d2=0 -> dist 0) and Qv[i, 28], then
splits both into bf16 hi/lo halves and stacks the three cross terms
(Qh.Ph + Qh.Pl + Ql.Ph) along the contraction axis: the PE's matmul cost is
N-cycles regardless of K, so one K=84 bf16 matmul gives fp32-grade d2
(validated: end-to-end loss error ~3e-8) at ~10x the speed of a native fp32
matmul (which runs as two half-rate passes).

Device (O(L^2) work): d2 = A^T.T @ B as K=84 bf16 matmuls (N=512 each, four
matmuls fill one 4-bank PSUM group tile), then per 2048-wide group:
clamp to [0, 100] on the vector engine (min(sqrt(d2), 10) == sqrt(min(d2, 100));
max(.,0) guards bf16-split cancellation), sqrt + free-axis sum fused on the
scalar engine.  Each of the 8 cores handles 256 frames and returns 256 per-frame
sums; the host applies the frame mask and final normalization.  eps inside the
sqrt is dropped: its contribution is O(1e-9) relative on this data.

Toolchain constraint: this walrus build allows ONE semaphore wait per
instruction.  The single fused input DMA (one queue semaphore) and the
no-reuse SBUF pools keep every compute instruction at <=1 wait; remaining
multi-wait instructions (the Tile exit drain) are split by _split_multi_waits.
"""

import sys

import numpy as np

for _p in ("/opt/trn_rl_repo",):
    if _p not in sys.path:
        sys.path.insert(0, _p)

import ml_dtypes
import concourse.bass as bass
import concourse.tile as tile
from concourse import mybir
from concourse.bass_utils import run_bass_kernel_spmd

L = 2048
N_CORES = 8
FRAMES_PER_CORE = L // N_CORES  # 256
NLJ = L * 3  # 6144
K = 28         # 7*8/2 upper-triangle pairs
KS = 3 * K + 1  # 85: three bf16 cross terms + one sqrt-guard bias row
N_CHUNK = 512
GROUP_CHUNKS = 4
GROUP_COLS = GROUP_CHUNKS * N_CHUNK  # 2048 = one 4-bank PSUM tile
N_GROUPS_LJ = NLJ // GROUP_COLS      # 3 groups of lj per frame tile
F_TILES = FRAMES_PER_CORE // 128     # 2
N_GROUPS = F_TILES * N_GROUPS_LJ     # 6
CLAMP = 10.0  # CLAMP_DISTANCE
# Sqrt-domain guard: d2 arrives as d2_true + BIAS*mask (the extra K-row), so
# bf16-split/fp32-accum cancellation error (|err| <~ 1e-4 on this data) can
# never push the sqrt argument negative, while masked columns stay exactly 0.
# Effect on the loss is ~5e-6 relative (measured).
BIAS = 1e-3

_PAIRS = [(a, b) for a in range(7) for b in range(a, 7)]


def _host_prep(pred_coords, true_coords, pred_rotation, pred_translation,
               true_rotation, true_translation, mask):
    """Return (B (84, 6144) bf16, A (L, 84) bf16): the stacked hi/lo splits of
    the quadratic-form factors.  All O(L) flops, float64."""
    pc = np.asarray(pred_coords, np.float64)
    tc = np.asarray(true_coords, np.float64)
    pR = np.asarray(pred_rotation, np.float64)
    pT = np.asarray(pred_translation, np.float64)
    tR = np.asarray(true_rotation, np.float64)
    tT = np.asarray(true_translation, np.float64)

    UT = np.concatenate([
        pc.reshape(L * 3, 3).T,
        tc.reshape(L * 3, 3).T,
        np.ones((1, L * 3)),
    ], axis=0)  # (7, 6144)

    offp = np.einsum('ic,idc->id', pT, pR)
    offt = np.einsum('ic,idc->id', tT, tR)
    W = np.concatenate([pR, -tR, -(offp - offt)[:, :, None]], axis=2)  # (L, 3, 7)
    Q = np.einsum('ida,idb->iab', W, W)  # (L, 7, 7)

    Qv = np.stack([Q[:, a, b] * (1.0 if a == b else 2.0) for (a, b) in _PAIRS],
                  axis=1).astype(np.float32)  # (L, 28)
    P = np.stack([UT[a] * UT[b] for (a, b) in _PAIRS], axis=0)  # (28, 6144)

    m_lj = np.repeat(np.asarray(mask, np.float64) != 0, 3)
    P32 = (P * m_lj[None, :]).astype(np.float32)

    def split(x):
        hi = x.astype(ml_dtypes.bfloat16)
        lo = (x - hi.astype(np.float32)).astype(ml_dtypes.bfloat16)
        return hi, lo

    Ph, Pl = split(P32)
    Qh, Ql = split(Qv)
    bias_row = (np.float32(BIAS) * m_lj.astype(np.float32))[None, :]
    B = np.concatenate([Ph, Pl, Ph,
                        bias_row.astype(ml_dtypes.bfloat16)], axis=0)  # (85, 6144)
    A = np.concatenate([Qh, Qh, Ql,
                        np.ones((L, 1), ml_dtypes.bfloat16)], axis=1)  # (L, 85)
    return np.ascontiguousarray(B), np.ascontiguousarray(A)


def _split_multi_waits(nc):
    """The TPB instruction encodings used by this walrus build carry a single
    semaphore wait.  Tile can emit several waits on one instruction (notably
    the kernel-tail drain).  Split the extras onto same-engine no-ops placed
    immediately before the instruction — engine-order execution makes this
    semantically identical."""
    for bbw in nc.main_func.blocks:
        il = bbw.instructions
        out = []
        changed = False
        for ins in il:
            si = ins.sync_info
            if si is not None and len(si.on_wait) > 1:
                waits = list(si.on_wait)
                for idx, w in enumerate(waits[:-1]):
                    out.append(mybir.InstNoOp(
                        name=f"{ins.name}-waitsplit{idx}",
                        engine=ins.engine,
                        sync_info=mybir.SyncInfo(on_wait=[w], on_update=[]),
                    ))
                si.on_wait = [waits[-1]]
                changed = True
            out.append(ins)
        if changed:
            bbw.instructions = out


def _build_program(split_waits=True):
    f32 = mybir.dt.float32
    bf16 = mybir.dt.bfloat16
    nc = bass.Bass()
    # Input layout: [Q (256) | lj block0 (2048) | block1 (2048) | block2 (2048)],
    # loaded by three DMAs (Q+block0, block1, block2) on distinct HW-DGE lanes
    # so compute on block0 overlaps the remaining transfers.
    inp = nc.declare_dram_parameter("inp", [KS, FRAMES_PER_CORE + NLJ], bf16,
                                    isOutput=False)
    # Raw per-group accumulators: cols 0:6 = sum(dist) (scalar engine),
    # cols 6:12 = sum(max(dist, 10)) (vector engine).  Host computes
    # sum(min(dist, 10)) = sum(dist) + 10*GROUP_COLS - sum(max(dist, 10)).
    fsums = nc.declare_dram_parameter("fsums", [128, 2 * N_GROUPS], f32,
                                      isOutput=True)
    Q0 = FRAMES_PER_CORE  # column where lj blocks start

    with tile.TileContext(nc) as tc:
        with tc.tile_pool(name="const", bufs=1) as const_pool, \
             tc.tile_pool(name="clamped", bufs=N_GROUPS) as clamped_pool, \
             tc.tile_pool(name="ps", bufs=2, space="PSUM") as ps:
            data = const_pool.tile([KS, FRAMES_PER_CORE + NLJ], bf16)
            # Block0 (+Q) arrives as four chunk-aligned DMAs so each of the
            # first four matmuls waits on exactly its own chunk and compute
            # starts ~2us after the first chunk lands; blocks 1 and 2 stream
            # in behind the compute.
            bounds = [0, Q0 + N_CHUNK, Q0 + 2 * N_CHUNK, Q0 + 3 * N_CHUNK,
                      Q0 + GROUP_COLS]
            for i in range(4):
                nc.sync.dma_start(data[:, bounds[i]:bounds[i + 1]],
                                  inp[:, bounds[i]:bounds[i + 1]])
            nc.sync.dma_start(data[:, Q0 + GROUP_COLS:Q0 + 2 * GROUP_COLS],
                              inp[:, Q0 + GROUP_COLS:Q0 + 2 * GROUP_COLS])
            nc.sync.dma_start(data[:, Q0 + 2 * GROUP_COLS:],
                              inp[:, Q0 + 2 * GROUP_COLS:])

            acc_a = const_pool.tile([128, N_GROUPS], f32)  # ACT: sum(dist)
            acc_d = const_pool.tile([128, N_GROUPS], f32)  # DVE: sum(excess)

            # Scalar-engine constant + two dummy activations: the sqrt bias
            # const-AP and the engine's own-semaphore ticks would otherwise
            # put a second wait on the first real sqrt (walrus allows one).
            bias_t = const_pool.tile([128, 1], f32)
            scratch_t = const_pool.tile([128, 1], f32)
            nc.scalar.memzero(bias_t[:])
            nc.scalar.activation(bias_t[:], bias_t[:],
                                 mybir.ActivationFunctionType.Sqrt,
                                 bias=bias_t[:, 0:1])
            nc.scalar.activation(scratch_t[:], bias_t[:],
                                 mybir.ActivationFunctionType.Sqrt,
                                 bias=bias_t[:, 0:1])

            # Group order is block-major so block-b compute overlaps the
            # DMA of block b+1.  g = b * F_TILES + f.
            for g in range(N_GROUPS):
                b = g // F_TILES
                f = g % F_TILES
                if f == 0 and b > 0:
                    # Standalone bf16 LDWEIGHTS as a pure wait-carrier: it
                    # absorbs block-b's DMA-queue wait on the PE so the real
                    # matmuls only ever wait on their PSUM-slot release
                    # (single-wait-per-instruction toolchain limit).
                    nc.tensor.ldweights(
                        data[:, Q0 + b * GROUP_COLS:Q0 + b * GROUP_COLS + 128])
                d2 = ps.tile([128, GROUP_COLS], f32)
                for c in range(GROUP_CHUNKS):
                    col = Q0 + b * GROUP_COLS + c * N_CHUNK
                    nc.tensor.matmul(
                        d2[:, c * N_CHUNK:(c + 1) * N_CHUNK],
                        data[:, f * 128:(f + 1) * 128],
                        data[:, col:col + N_CHUNK],
                        start=True, stop=True,
                    )
                # sqrt straight from PSUM on the scalar engine (faster than
                # the vector engine there per the TRN2 errata tables), with
                # the free-axis sum fused; the guard row keeps d2 >= 0.
                dist = clamped_pool.tile([128, GROUP_COLS], f32)
                nc.scalar.activation(
                    dist[:], d2[:], mybir.ActivationFunctionType.Sqrt,
                    bias=bias_t[:, 0:1],
                    accum_out=acc_a[:, g:g + 1],
                )
                # clamp via min(y,10) = y + 10 - max(y,10) summed exactly:
                # acc_d = sum(max(dist, 10)); host adds the 10*N constant.
                # Single-source fp32 SBUF tensor_scalar (op1 is the
                # reduction op when accum_out is given) runs in 2x mode.
                nc.vector.tensor_scalar(
                    out=dist[:], in0=dist[:],
                    scalar1=CLAMP, scalar2=None,
                    op0=mybir.AluOpType.max, op1=mybir.AluOpType.add,
                    accum_out=acc_d[:, g:g + 1],
                )

            # Two fresh HW-DGE lanes: each output DMA carries one wait.
            nc.sync.dma_start(fsums[:, 0:N_GROUPS], acc_a[:])
            nc.sync.dma_start(fsums[:, N_GROUPS:2 * N_GROUPS], acc_d[:])
    if split_waits:
        # Needed for the walrus compile; CoreSim can't model the raw no-ops.
        _split_multi_waits(nc)
    return nc


def kernel(pred_coords, true_coords, pred_rotation, pred_translation,
           true_rotation, true_translation, mask, **_run_kwargs):
    mask = np.asarray(mask)
    B, A = _host_prep(pred_coords, true_coords, pred_rotation,
                      pred_translation, true_rotation, true_translation, mask)

    in_maps = []
    for c in range(N_CORES):
        a_c = A[c * FRAMES_PER_CORE:(c + 1) * FRAMES_PER_CORE].T  # (84, 256)
        in_maps.append({"inp": np.ascontiguousarray(
            np.concatenate([a_c, B], axis=1))})  # (84, 6400)

    nc = _build_program()
    res = run_bass_kernel_spmd(nc, in_maps, list(range(N_CORES)),
                               **_run_kwargs)

    m_i = np.asarray(mask, np.float64)
    numer = 0.0
    for c in range(N_CORES):
        fs = np.asarray(res.results[c]["fsums"], np.float64)  # (128, 12)
        # sum(min(dist,10)) = sum(dist) + 10*N - sum(max(dist,10)) per group
        clamped = fs[:, :N_GROUPS] + CLAMP * GROUP_COLS - fs[:, N_GROUPS:]
        # acc column g = b * F_TILES + f; frame index = c*256 + f*128 + p
        frame_sums = clamped.reshape(128, 3, F_TILES).sum(axis=1).T.reshape(-1)
        numer += float((m_i[c * FRAMES_PER_CORE:(c + 1) * FRAMES_PER_CORE]
                        * frame_sums).sum())

    denom = float(m_i.sum()) ** 2 * 3.0 + 1e-8
    out = np.float32(numer / denom / 10.0)
    if _run_kwargs:
        return out, res
    return out


# revision 29
# speedup vs baseline: 1.0727x; 1.0727x over previous
"""FAPE loss kernel for Trainium2 (8 NeuronCores, Bass/Tile).

Math
----
The reference computes, for frames i and residue-atoms (l, j):

    local[i, lj, d] = sum_c coords[lj, c] * R[i, d, c] - off[i, d]
    d2[i, lj]       = sum_d (pred_local - true_local)^2
    loss            = sum_{i,lj} m[i] * m[l] * min(sqrt(d2 + eps), 10) / ((sum m)^2 * 3 + eps) / 10

The delta is linear in the 7-vector u'[lj] = [pred_coords(3), true_coords(3), 1]:
    delta_d[i, lj] = dot(u'[lj], w_d[i]),  w_d[i] = [pR[i,d,:], -tR[i,d,:], -(offp-offt)[i,d]]
so d2 is a quadratic form
    d2[i, lj] = sum_{a<=b} mult_ab * u'_a u'_b * Q[i,(a,b)],  Q[i] = sum_d w_d w_d^T

Host (O(L) work): builds P[28, 6144] = pairwise products of u' (residue mask folded
in as zeroed columns, so masked entries give d2=0 -> dist 0) and Qv[i, 28], then
splits both into bf16 hi/lo halves and stacks the three cross terms
(Qh.Ph + Qh.Pl + Ql.Ph) along the contraction axis: the PE's matmul cost is
N-cycles regardless of K, so one K=84 bf16 matmul gives fp32-grade d2
(validated: end-to-end loss error ~3e-8) at ~10x the speed of a native fp32
matmul (which runs as two half-rate passes).

Device (O(L^2) work): d2 = A^T.T @ B as K=84 bf16 matmuls (N=512 each, four
matmuls fill one 4-bank PSUM group tile), then per 2048-wide group:
clamp to [0, 100] on the vector engine (min(sqrt(d2), 10) == sqrt(min(d2, 100));
max(.,0) guards bf16-split cancellation), sqrt + free-axis sum fused on the
scalar engine.  Each of the 8 cores handles 256 frames and returns 256 per-frame
sums; the host applies the frame mask and final normalization.  eps inside the
sqrt is dropped: its contribution is O(1e-9) relative on this data.

Toolchain constraint: this walrus build allows ONE semaphore wait per
instruction.  The single fused input DMA (one queue semaphore) and the
no-reuse SBUF pools keep every compute instruction at <=1 wait; remaining
multi-wait instructions (the Tile exit drain) are split by _split_multi_waits.
"""

import sys

import numpy as np

for _p in ("/opt/trn_rl_repo",):
    if _p not in sys.path:
        sys.path.insert(0, _p)

import ml_dtypes
import concourse.bass as bass
import concourse.tile as tile
from concourse import mybir
from concourse.bass_utils import run_bass_kernel_spmd

L = 2048
N_CORES = 8
FRAMES_PER_CORE = L // N_CORES  # 256
NLJ = L * 3  # 6144
K = 28         # 7*8/2 upper-triangle pairs
KS = 3 * K     # 84: three bf16 cross terms stacked on the contraction axis
N_CHUNK = 512
GROUP_CHUNKS = 4
GROUP_COLS = GROUP_CHUNKS * N_CHUNK  # 2048 = one 4-bank PSUM tile
N_GROUPS_LJ = NLJ // GROUP_COLS      # 3 groups of lj per frame tile
F_TILES = FRAMES_PER_CORE // 128     # 2
N_GROUPS = F_TILES * N_GROUPS_LJ     # 6
CLAMP2 = 100.0  # CLAMP_DISTANCE ** 2

_PAIRS = [(a, b) for a in range(7) for b in range(a, 7)]


def _host_prep(pred_coords, true_coords, pred_rotation, pred_translation,
               true_rotation, true_translation, mask):
    """Return (B (84, 6144) bf16, A (L, 84) bf16): the stacked hi/lo splits of
    the quadratic-form factors.  All O(L) flops, float64."""
    pc = np.asarray(pred_coords, np.float64)
    tc = np.asarray(true_coords, np.float64)
    pR = np.asarray(pred_rotation, np.float64)
    pT = np.asarray(pred_translation, np.float64)
    tR = np.asarray(true_rotation, np.float64)
    tT = np.asarray(true_translation, np.float64)

    UT = np.concatenate([
        pc.reshape(L * 3, 3).T,
        tc.reshape(L * 3, 3).T,
        np.ones((1, L * 3)),
    ], axis=0)  # (7, 6144)

    offp = np.einsum('ic,idc->id', pT, pR)
    offt = np.einsum('ic,idc->id', tT, tR)
    W = np.concatenate([pR, -tR, -(offp - offt)[:, :, None]], axis=2)  # (L, 3, 7)
    Q = np.einsum('ida,idb->iab', W, W)  # (L, 7, 7)

    Qv = np.stack([Q[:, a, b] * (1.0 if a == b else 2.0) for (a, b) in _PAIRS],
                  axis=1).astype(np.float32)  # (L, 28)
    P = np.stack([UT[a] * UT[b] for (a, b) in _PAIRS], axis=0)  # (28, 6144)

    m_lj = np.repeat(np.asarray(mask, np.float64) != 0, 3)
    P32 = (P * m_lj[None, :]).astype(np.float32)

    def split(x):
        hi = x.astype(ml_dtypes.bfloat16)
        lo = (x - hi.astype(np.float32)).astype(ml_dtypes.bfloat16)
        return hi, lo

    Ph, Pl = split(P32)
    Qh, Ql = split(Qv)
    B = np.concatenate([Ph, Pl, Ph], axis=0)   # (84, 6144)
    A = np.concatenate([Qh, Qh, Ql], axis=1)   # (L, 84)
    return np.ascontiguousarray(B), np.ascontiguousarray(A)


def _split_multi_waits(nc):
    """The TPB instruction encodings used by this walrus build carry a single
    semaphore wait.  Tile can emit several waits on one instruction (notably
    the kernel-tail drain).  Split the extras onto same-engine no-ops placed
    immediately before the instruction — engine-order execution makes this
    semantically identical."""
    for bbw in nc.main_func.blocks:
        il = bbw.instructions
        out = []
        changed = False
        for ins in il:
            si = ins.sync_info
            if si is not None and len(si.on_wait) > 1:
                waits = list(si.on_wait)
                for idx, w in enumerate(waits[:-1]):
                    out.append(mybir.InstNoOp(
                        name=f"{ins.name}-waitsplit{idx}",
                        engine=ins.engine,
                        sync_info=mybir.SyncInfo(on_wait=[w], on_update=[]),
                    ))
                si.on_wait = [waits[-1]]
                changed = True
            out.append(ins)
        if changed:
            bbw.instructions = out


def _build_program(split_waits=True):
    f32 = mybir.dt.float32
    bf16 = mybir.dt.bfloat16
    nc = bass.Bass()
    # Input layout: [Q (256) | lj block0 (2048) | block1 (2048) | block2 (2048)],
    # loaded by three DMAs (Q+block0, block1, block2) on distinct HW-DGE lanes
    # so compute on block0 overlaps the remaining transfers.
    inp = nc.declare_dram_parameter("inp", [KS, FRAMES_PER_CORE + NLJ], bf16,
                                    isOutput=False)
    # Raw per-group accumulator; host folds the 6 columns into frame sums.
    fsums = nc.declare_dram_parameter("fsums", [128, N_GROUPS], f32, isOutput=True)
    Q0 = FRAMES_PER_CORE  # column where lj blocks start

    with tile.TileContext(nc) as tc:
        with tc.tile_pool(name="const", bufs=1) as const_pool, \
             tc.tile_pool(name="clamped", bufs=N_GROUPS) as clamped_pool, \
             tc.tile_pool(name="ps", bufs=2, space="PSUM") as ps:
            data = const_pool.tile([KS, FRAMES_PER_CORE + NLJ], bf16)
            # Block0 (+Q) arrives as four chunk-aligned DMAs so each of the
            # first four matmuls waits on exactly its own chunk and compute
            # starts ~2us after the first chunk lands; blocks 1 and 2 stream
            # in behind the compute.
            bounds = [0, Q0 + N_CHUNK, Q0 + 2 * N_CHUNK, Q0 + 3 * N_CHUNK,
                      Q0 + GROUP_COLS, Q0 + 2 * GROUP_COLS, Q0 + 3 * GROUP_COLS]
            # Alternate the two HW-DGE rings (SP and ACT sequencers): DMA
            # issue costs ~0.7us on the issuing engine, so splitting the six
            # issues across two engines halves the serial issue latency.
            engines = [nc.sync, nc.scalar, nc.sync, nc.scalar, nc.sync, nc.scalar]
            for i in range(6):
                engines[i].dma_start(data[:, bounds[i]:bounds[i + 1]],
                                     inp[:, bounds[i]:bounds[i + 1]])

            acc = const_pool.tile([128, N_GROUPS], f32)

            # Scalar-engine constant + two dummy activations: the sqrt bias
            # const-AP and the engine's own-semaphore ticks would otherwise
            # put a second wait on the first real sqrt (walrus allows one).
            bias_t = const_pool.tile([128, 1], f32)
            scratch_t = const_pool.tile([128, 1], f32)
            nc.scalar.memzero(bias_t[:])
            nc.scalar.activation(bias_t[:], bias_t[:],
                                 mybir.ActivationFunctionType.Sqrt,
                                 bias=bias_t[:, 0:1])
            nc.scalar.activation(scratch_t[:], bias_t[:],
                                 mybir.ActivationFunctionType.Sqrt,
                                 bias=bias_t[:, 0:1])

            # Group order is block-major so block-b compute overlaps the
            # DMA of block b+1.  g = b * F_TILES + f.
            for g in range(N_GROUPS):
                b = g // F_TILES
                f = g % F_TILES
                if f == 0 and b > 0:
                    # Standalone bf16 LDWEIGHTS as a pure wait-carrier: it
                    # absorbs block-b's DMA-queue wait on the PE so the real
                    # matmuls only ever wait on their PSUM-slot release
                    # (single-wait-per-instruction toolchain limit).
                    nc.tensor.ldweights(
                        data[:, Q0 + b * GROUP_COLS:Q0 + b * GROUP_COLS + 128])
                d2 = ps.tile([128, GROUP_COLS], f32)
                for c in range(GROUP_CHUNKS):
                    col = Q0 + b * GROUP_COLS + c * N_CHUNK
                    nc.tensor.matmul(
                        d2[:, c * N_CHUNK:(c + 1) * N_CHUNK],
                        data[:, f * 128:(f + 1) * 128],
                        data[:, col:col + N_CHUNK],
                        start=True, stop=True,
                    )
                clamped = clamped_pool.tile([128, GROUP_COLS], f32)
                nc.vector.tensor_scalar(
                    out=clamped[:], in0=d2[:],
                    scalar1=0.0, scalar2=CLAMP2,
                    op0=mybir.AluOpType.max, op1=mybir.AluOpType.min,
                )
                nc.scalar.activation(
                    clamped[:], clamped[:], mybir.ActivationFunctionType.Sqrt,
                    bias=bias_t[:, 0:1],
                    accum_out=acc[:, g:g + 1],
                )

            # Fresh HW-DGE lane: single data-ready wait.
            nc.sync.dma_start(fsums[:], acc[:])
    if split_waits:
        # Needed for the walrus compile; CoreSim can't model the raw no-ops.
        _split_multi_waits(nc)
    return nc


def kernel(pred_coords, true_coords, pred_rotation, pred_translation,
           true_rotation, true_translation, mask, **_run_kwargs):
    mask = np.asarray(mask)
    B, A = _host_prep(pred_coords, true_coords, pred_rotation,
                      pred_translation, true_rotation, true_translation, mask)

    in_maps = []
    for c in range(N_CORES):
        a_c = A[c * FRAMES_PER_CORE:(c + 1) * FRAMES_PER_CORE].T  # (84, 256)
        in_maps.append({"inp": np.ascontiguousarray(
            np.concatenate([a_c, B], axis=1))})  # (84, 6400)

    nc = _build_program()
    res = run_bass_kernel_spmd(nc, in_maps, list(range(N_CORES)),
                               **_run_kwargs)

    m_i = np.asarray(mask, np.float64)
    numer = 0.0
    for c in range(N_CORES):
        fs = np.asarray(res.results[c]["fsums"], np.float64)  # (128, N_GROUPS)
        # acc column g = b * F_TILES + f; frame index = c*256 + f*128 + p
        frame_sums = fs.reshape(128, 3, F_TILES).sum(axis=1).T.reshape(-1)
        numer += float((m_i[c * FRAMES_PER_CORE:(c + 1) * FRAMES_PER_CORE]
                        * frame_sums).sum())

    denom = float(m_i.sum()) ** 2 * 3.0 + 1e-8
    out = np.float32(numer / denom / 10.0)
    if _run_kwargs:
        return out, res
    return out
